# revision 14
# baseline (speedup 1.0000x reference)
"""Trainium2 Bass kernel for CorrectedPartialCharges.

out[i] = pc[i] + (total_charge[g] - seg_sum[g]) / n_atoms[g],  g = i // 256

Sharding: graphs are data-parallel across the 8 cores (4096 graphs /
1,048,576 atoms per core); segment sums and the gather-broadcast stay
device-local. On each core, partition p owns 32 contiguous graphs.

Wire format is bf16 (the 2e-2 rel-err budget allows it): node charges are
rounded to bf16 on the host, halving HBM traffic both ways; all device
accumulation is fp32. total_charge is pre-divided by 256 on the host so
the leftover is one fused scalar_tensor_tensor op.

Engine split (per [128, 2048] tile, k=8 graphs per partition):
  - Tensor: 4 accumulating identity matmuls fold each graph's 256 atoms
    into 64 PSUM columns (the bulk of the segment reduction, on an
    otherwise idle engine).
  - Vector: 1x reduce of the [128, k, 64] PSUM partials -> seg, fused
    leftover, and most per-graph tensor_scalar adds.
  - Scalar/ACT: per-graph bias adds for a subset of j-indices... no --
    whole-tile granularity (the tile dependency tracker serializes
    cross-engine writes to one tile), so ACT owns the adds of designated
    tiles and Vector the rest.
  - GpSimd: only the small identity + total-charge DMAs (SWDGE); its
    tensor ops have ~4.7us dispatch overhead and are useless here.
DMA: loads/stores alternate between the two HWDGE rings; the last store
is split across both rings to shorten the drain tail.
"""

import ml_dtypes
import numpy as np

import concourse.bacc as bacc
import concourse.bass as bass  # noqa: F401
import concourse.mybir as mybir
import concourse.tile as tile
from concourse.bass_utils import run_bass_kernel_spmd

N_CORES = 8
ATOMS_PER_GRAPH = 256
N_GRAPHS = 32768
N_ATOMS = N_GRAPHS * ATOMS_PER_GRAPH
P = 128

G_PER_CORE = N_GRAPHS // N_CORES          # 4096 graphs per core
A_PER_CORE = G_PER_CORE * ATOMS_PER_GRAPH  # 1,048,576 atoms per core

# Knobs read by test.py when experimenting.
# Tile widths by index (atoms per partition); tiles are contiguous spans of
# the free dim in index order. Sum must be 8192; each a multiple of 256 with
# width/256 <= 8 (PSUM accumulation bank limit).
TILE_W = (2048, 2048, 1024, 1024, 2048)
PSUM_W = 64               # columns per graph after the matmul pre-reduce
RED_MODE = "matmul"       # "matmul" | "halve"
TILE_ORDER = (2, 3, 0, 1, 4)  # processing order; first listed loads first
# add engine per tile ("vector" or "scalar"); ACT tiles should be early in
# TILE_ORDER so the ACT chain starts as soon as possible.
TILE_ADD_ENGINE = {2: "scalar", 3: "scalar", 0: "vector", 1: "vector", 4: "vector"}
SPLIT_LAST_STORE = True

_TRACE = False
_TRACE_KWARGS = {}


def _build(tile_w=None, order=None, add_eng=None, red_mode=None, psum_w=None,
           split_last=None):
    tile_w = TILE_W if tile_w is None else tile_w
    order = TILE_ORDER if order is None else order
    add_eng = TILE_ADD_ENGINE if add_eng is None else add_eng
    red_mode = RED_MODE if red_mode is None else red_mode
    psum_w = PSUM_W if psum_w is None else psum_w
    split_last = SPLIT_LAST_STORE if split_last is None else split_last

    nt = len(tile_w)
    ap_free = A_PER_CORE // P     # 8192 atoms per partition
    gp = G_PER_CORE // P          # 32 graphs per partition
    half = ATOMS_PER_GRAPH // 2
    n_pass = ATOMS_PER_GRAPH // psum_w
    offs = [0]
    for w_ in tile_w:
        offs.append(offs[-1] + w_)
    assert offs[-1] == ap_free
    for w_ in tile_w:
        assert w_ % ATOMS_PER_GRAPH == 0
        assert (w_ // ATOMS_PER_GRAPH) * psum_w * 4 <= 2048, \
            "psum accumulation group must fit one bank"
    assert tuple(sorted(order)) == tuple(range(nt))

    nc = bacc.Bacc(None, target_bir_lowering=False, enable_partition_id=False)

    pc = nc.dram_tensor("pc", [A_PER_CORE], mybir.dt.bfloat16, kind="ExternalInput")
    # total_charge / 256, fp32
    tcs = nc.dram_tensor("tcs", [G_PER_CORE], mybir.dt.float32, kind="ExternalInput")
    eye = nc.dram_tensor("eye", [P * P], mybir.dt.bfloat16, kind="ExternalInput")
    out = nc.dram_tensor("out", [A_PER_CORE], mybir.dt.bfloat16, kind="ExternalOutput")

    pc_v = pc[:].rearrange("(p n) -> p n", p=P)
    out_v = out[:].rearrange("(p n) -> p n", p=P)
    tcs_v = tcs[:].rearrange("(p k) -> p k", p=P)
    eye_v = eye[:].rearrange("(p n) -> p n", p=P)

    with tile.TileContext(nc) as tc:
        with (
            tc.tile_pool(name="io", bufs=nt) as io_pool,
            tc.tile_pool(name="half", bufs=2) as half_pool,
            tc.tile_pool(name="small", bufs=2 * nt) as small_pool,
            tc.tile_pool(name="consts", bufs=1) as const_pool,
            tc.tile_pool(name="psum", bufs=min(nt, 6), space="PSUM") as psum_pool,
        ):
            # One tiny constant at the head of each HWDGE ring (one small DMA
            # per ring costs ~1 inter-DMA bubble; two would cost two). They
            # must not ride the SWDGE queue, whose packets round-robin with
            # (and finish after) all load traffic.
            eye_tile = const_pool.tile([P, P], mybir.dt.bfloat16, tag="eye")
            nc.sync.dma_start(out=eye_tile[:], in_=eye_v)
            tc_tile = const_pool.tile([P, gp], mybir.dt.float32, tag="tc")
            nc.scalar.dma_start(out=tc_tile[:], in_=tcs_v)

            # Queue every input tile load up front, alternating HWDGE rings.
            xs = {}
            for i, t in enumerate(order):
                w_ = tile_w[t]
                x = io_pool.tile([P, w_], mybir.dt.bfloat16, tag="x")
                eng = nc.sync if i % 2 == 0 else nc.scalar
                eng.dma_start(out=x[:], in_=pc_v[:, offs[t] : offs[t] + w_])
                xs[t] = x

            goff = [o // ATOMS_PER_GRAPH for o in offs]  # graph offsets
            for i, t in enumerate(order):
                x = xs[t]
                w_ = tile_w[t]
                k = w_ // ATOMS_PER_GRAPH
                x3 = x[:].rearrange("p (k a) -> p k a", a=ATOMS_PER_GRAPH)

                seg = small_pool.tile([P, k], mybir.dt.float32, tag="seg")
                if red_mode == "matmul":
                    # Fold 256 atoms -> psum_w columns per graph with
                    # accumulating identity matmuls on the Tensor engine.
                    ps = psum_pool.tile([P, k, psum_w], mybir.dt.float32, tag="ps")
                    for s in range(n_pass):
                        nc.tensor.matmul(
                            ps[:],
                            eye_tile[:],
                            x3[:, :, s * psum_w : (s + 1) * psum_w],
                            start=(s == 0),
                            stop=(s == n_pass - 1),
                        )
                    nc.vector.reduce_sum(
                        out=seg[:], in_=ps[:], axis=mybir.AxisListType.X
                    )
                else:
                    u = half_pool.tile([P, k * half], mybir.dt.bfloat16, tag="u")
                    u3 = u[:].rearrange("p (k a) -> p k a", a=half)
                    nc.vector.tensor_add(
                        out=u3, in0=x3[:, :, 0:half],
                        in1=x3[:, :, half:ATOMS_PER_GRAPH],
                    )
                    nc.vector.reduce_sum(out=seg[:], in_=u3, axis=mybir.AxisListType.X)

                # left = (seg * -1/256) + tc/256   (fused)
                left = small_pool.tile([P, k], mybir.dt.float32, tag="left")
                nc.vector.scalar_tensor_tensor(
                    out=left[:],
                    in0=seg[:],
                    scalar=-1.0 / ATOMS_PER_GRAPH,
                    in1=tc_tile[:, goff[t] : goff[t] + k],
                    op0=mybir.AluOpType.mult,
                    op1=mybir.AluOpType.add,
                )

                for j in range(k):
                    blk = x[:, j * ATOMS_PER_GRAPH : (j + 1) * ATOMS_PER_GRAPH]
                    if add_eng[t] == "scalar":
                        nc.scalar.add(out=blk, in_=blk, add=left[:, j : j + 1])
                    else:
                        nc.vector.tensor_scalar_add(
                            out=blk, in0=blk, scalar1=left[:, j : j + 1]
                        )
                if split_last and i == nt - 1:
                    hw_ = w_ // 2
                    nc.scalar.dma_start(
                        out=out_v[:, offs[t] : offs[t] + hw_], in_=x[:, 0:hw_]
                    )
                    nc.sync.dma_start(
                        out=out_v[:, offs[t] + hw_ : offs[t] + w_], in_=x[:, hw_:w_]
                    )
                else:
                    # stores for ACT-add tiles go on sync so they don't queue
                    # behind the ACT add chain; vector tiles use the ACT ring.
                    eng = nc.sync if add_eng[t] == "scalar" else nc.scalar
                    eng.dma_start(out=out_v[:, offs[t] : offs[t] + w_], in_=x[:])

    nc.finalize()
    return nc


_NC_CACHE = {}


def _get_nc():
    key = (TILE_W, TILE_ORDER, tuple(sorted(TILE_ADD_ENGINE.items())), RED_MODE,
           PSUM_W, SPLIT_LAST_STORE)
    if key not in _NC_CACHE:
        _NC_CACHE[key] = _build()
    return _NC_CACHE[key]


def _cpu_fallback(pc, total_charge, batch, n_atoms):
    num_segments = n_atoms.shape[0]
    seg = np.bincount(batch, weights=pc.astype(np.float64), minlength=num_segments)
    leftover = (total_charge - seg.astype(np.float32)) / n_atoms.astype(np.float32)
    return (pc + leftover[batch]).astype(np.float32)


_EYE = None


def kernel(**inputs) -> np.ndarray:
    global _EYE
    pc = np.ascontiguousarray(
        np.asarray(inputs["node_outputs"], dtype=np.float32).reshape(-1)
    )
    total_charge = np.ascontiguousarray(
        np.asarray(inputs["total_charge"], dtype=np.float32).reshape(-1)
    )
    batch = np.asarray(inputs["batch"]).reshape(-1)
    n_atoms = np.ascontiguousarray(np.asarray(inputs["n_atoms"], dtype=np.int32).reshape(-1))

    # The device kernel hardcodes the uniform 256-atoms-per-graph layout the
    # reference generator produces; anything else goes through numpy.
    if (
        pc.shape[0] != N_ATOMS
        or total_charge.shape[0] != N_GRAPHS
        or not np.all(n_atoms == ATOMS_PER_GRAPH)
        or not np.array_equal(
            batch.astype(np.int64),
            np.arange(N_ATOMS, dtype=np.int64) // ATOMS_PER_GRAPH,
        )
    ):
        return _cpu_fallback(pc, total_charge, batch, n_atoms)

    pc_b = pc.astype(ml_dtypes.bfloat16)
    tcs = (total_charge * (1.0 / ATOMS_PER_GRAPH)).astype(np.float32)
    if _EYE is None:
        _EYE = np.eye(P, dtype=ml_dtypes.bfloat16).reshape(-1)

    nc = _get_nc()
    in_maps = []
    for c in range(N_CORES):
        in_maps.append(
            {
                "pc": pc_b[c * A_PER_CORE : (c + 1) * A_PER_CORE],
                "tcs": tcs[c * G_PER_CORE : (c + 1) * G_PER_CORE],
                "eye": _EYE,
            }
        )
    res = run_bass_kernel_spmd(
        nc, in_maps, list(range(N_CORES)), trace=_TRACE, **_TRACE_KWARGS
    )
    out = np.concatenate([r["out"] for r in res.results]).astype(np.float32)
    if _TRACE:
        kernel.last_results = res
    return out


# revision 16
# speedup vs baseline: 1.0033x; 1.0033x over previous
"""Trainium2 Bass kernel for CorrectedPartialCharges.

out[i] = pc[i] + (total_charge[g] - seg_sum[g]) / n_atoms[g],  g = i // 256

Sharding: graphs are data-parallel across the 8 cores (4096 graphs /
1,048,576 atoms per core); segment sums and the gather-broadcast stay
device-local. On each core, partition p owns 32 contiguous graphs.

Wire format is bf16 (the 2e-2 rel-err budget allows it): node charges are
rounded to bf16 on the host, halving HBM traffic both ways; all device
accumulation is fp32. total_charge is pre-divided by 256 on the host so
the leftover is one fused scalar_tensor_tensor op.

Engine split (per [128, 2048] tile, k=8 graphs per partition):
  - Tensor: 4 accumulating identity matmuls fold each graph's 256 atoms
    into 64 PSUM columns (the bulk of the segment reduction, on an
    otherwise idle engine).
  - Vector: 1x reduce of the [128, k, 64] PSUM partials -> seg, fused
    leftover, and most per-graph tensor_scalar adds.
  - Scalar/ACT: per-graph bias adds for a subset of j-indices... no --
    whole-tile granularity (the tile dependency tracker serializes
    cross-engine writes to one tile), so ACT owns the adds of designated
    tiles and Vector the rest.
  - GpSimd: only the small identity + total-charge DMAs (SWDGE); its
    tensor ops have ~4.7us dispatch overhead and are useless here.
DMA: loads/stores alternate between the two HWDGE rings; the last store
is split across both rings to shorten the drain tail.
"""

import ml_dtypes
import numpy as np

import concourse.bacc as bacc
import concourse.bass as bass  # noqa: F401
import concourse.mybir as mybir
import concourse.tile as tile
from concourse.bass_utils import run_bass_kernel_spmd

N_CORES = 8
ATOMS_PER_GRAPH = 256
N_GRAPHS = 32768
N_ATOMS = N_GRAPHS * ATOMS_PER_GRAPH
P = 128

G_PER_CORE = N_GRAPHS // N_CORES          # 4096 graphs per core
A_PER_CORE = G_PER_CORE * ATOMS_PER_GRAPH  # 1,048,576 atoms per core

# Knobs read by test.py when experimenting.
# Tile widths by index (atoms per partition); tiles are contiguous spans of
# the free dim in index order. Sum must be 8192; each a multiple of 256 with
# width/256 <= 8 (PSUM accumulation bank limit).
TILE_W = (1024, 1024, 2048, 2048, 2048)
PSUM_W = 64               # columns per graph after the matmul pre-reduce
RED_MODE = "matmul"       # "matmul" | "halve"
TILE_ORDER = (0, 1, 4, 2, 3)  # processing order
# load queue per tile: sync/scalar rings carry [tiny const, small tile, big
# tile] each (>=2 DMAs per ring costs ~1.5us inter-DMA bubble apiece, so
# keep queues short); the SWDGE (gpsimd) queue carries one big load that
# streams with no queue head and lands first among the big tiles.
TILE_LOAD_Q = {0: "sync", 1: "scalar", 4: "gpsimd", 2: "sync", 3: "scalar"}
# add engine per tile ("vector" or "scalar"); ACT tiles should be early in
# TILE_ORDER so the ACT chain starts as soon as possible.
TILE_ADD_ENGINE = {0: "scalar", 1: "scalar", 4: "vector", 2: "vector", 3: "vector"}
SPLIT_LAST_STORE = True

_TRACE = False
_TRACE_KWARGS = {}


def _build(tile_w=None, order=None, add_eng=None, red_mode=None, psum_w=None,
           split_last=None):
    tile_w = TILE_W if tile_w is None else tile_w
    order = TILE_ORDER if order is None else order
    add_eng = TILE_ADD_ENGINE if add_eng is None else add_eng
    red_mode = RED_MODE if red_mode is None else red_mode
    psum_w = PSUM_W if psum_w is None else psum_w
    split_last = SPLIT_LAST_STORE if split_last is None else split_last

    nt = len(tile_w)
    ap_free = A_PER_CORE // P     # 8192 atoms per partition
    gp = G_PER_CORE // P          # 32 graphs per partition
    half = ATOMS_PER_GRAPH // 2
    n_pass = ATOMS_PER_GRAPH // psum_w
    offs = [0]
    for w_ in tile_w:
        offs.append(offs[-1] + w_)
    assert offs[-1] == ap_free
    for w_ in tile_w:
        assert w_ % ATOMS_PER_GRAPH == 0
        assert (w_ // ATOMS_PER_GRAPH) * psum_w * 4 <= 2048, \
            "psum accumulation group must fit one bank"
    assert tuple(sorted(order)) == tuple(range(nt))

    nc = bacc.Bacc(None, target_bir_lowering=False, enable_partition_id=False)

    pc = nc.dram_tensor("pc", [A_PER_CORE], mybir.dt.bfloat16, kind="ExternalInput")
    # total_charge / 256, fp32
    tcs = nc.dram_tensor("tcs", [G_PER_CORE], mybir.dt.float32, kind="ExternalInput")
    eye = nc.dram_tensor("eye", [P * P], mybir.dt.bfloat16, kind="ExternalInput")
    out = nc.dram_tensor("out", [A_PER_CORE], mybir.dt.bfloat16, kind="ExternalOutput")

    pc_v = pc[:].rearrange("(p n) -> p n", p=P)
    out_v = out[:].rearrange("(p n) -> p n", p=P)
    tcs_v = tcs[:].rearrange("(p k) -> p k", p=P)
    eye_v = eye[:].rearrange("(p n) -> p n", p=P)

    with tile.TileContext(nc) as tc:
        with (
            tc.tile_pool(name="io", bufs=nt) as io_pool,
            tc.tile_pool(name="half", bufs=2) as half_pool,
            tc.tile_pool(name="small", bufs=2 * nt) as small_pool,
            tc.tile_pool(name="consts", bufs=1) as const_pool,
            tc.tile_pool(name="psum", bufs=min(nt, 6), space="PSUM") as psum_pool,
        ):
            # One tiny constant at the head of each HWDGE ring (one small DMA
            # per ring costs ~1 inter-DMA bubble; two would cost two). They
            # must not ride the SWDGE queue, whose packets round-robin with
            # (and finish after) all load traffic.
            eye_tile = const_pool.tile([P, P], mybir.dt.bfloat16, tag="eye")
            nc.sync.dma_start(out=eye_tile[:], in_=eye_v)
            tc_tile = const_pool.tile([P, gp], mybir.dt.float32, tag="tc")
            nc.scalar.dma_start(out=tc_tile[:], in_=tcs_v)

            # Queue every input tile load up front on its assigned queue.
            xs = {}
            for i, t in enumerate(order):
                w_ = tile_w[t]
                x = io_pool.tile([P, w_], mybir.dt.bfloat16, tag="x")
                eng = getattr(nc, TILE_LOAD_Q[t])
                eng.dma_start(out=x[:], in_=pc_v[:, offs[t] : offs[t] + w_])
                xs[t] = x

            goff = [o // ATOMS_PER_GRAPH for o in offs]  # graph offsets
            for i, t in enumerate(order):
                x = xs[t]
                w_ = tile_w[t]
                k = w_ // ATOMS_PER_GRAPH
                x3 = x[:].rearrange("p (k a) -> p k a", a=ATOMS_PER_GRAPH)

                seg = small_pool.tile([P, k], mybir.dt.float32, tag="seg")
                if red_mode == "matmul":
                    # Fold 256 atoms -> psum_w columns per graph with
                    # accumulating identity matmuls on the Tensor engine.
                    ps = psum_pool.tile([P, k, psum_w], mybir.dt.float32, tag="ps")
                    for s in range(n_pass):
                        nc.tensor.matmul(
                            ps[:],
                            eye_tile[:],
                            x3[:, :, s * psum_w : (s + 1) * psum_w],
                            start=(s == 0),
                            stop=(s == n_pass - 1),
                        )
                    nc.vector.reduce_sum(
                        out=seg[:], in_=ps[:], axis=mybir.AxisListType.X
                    )
                else:
                    u = half_pool.tile([P, k * half], mybir.dt.bfloat16, tag="u")
                    u3 = u[:].rearrange("p (k a) -> p k a", a=half)
                    nc.vector.tensor_add(
                        out=u3, in0=x3[:, :, 0:half],
                        in1=x3[:, :, half:ATOMS_PER_GRAPH],
                    )
                    nc.vector.reduce_sum(out=seg[:], in_=u3, axis=mybir.AxisListType.X)

                # left = (seg * -1/256) + tc/256   (fused)
                left = small_pool.tile([P, k], mybir.dt.float32, tag="left")
                nc.vector.scalar_tensor_tensor(
                    out=left[:],
                    in0=seg[:],
                    scalar=-1.0 / ATOMS_PER_GRAPH,
                    in1=tc_tile[:, goff[t] : goff[t] + k],
                    op0=mybir.AluOpType.mult,
                    op1=mybir.AluOpType.add,
                )

                for j in range(k):
                    blk = x[:, j * ATOMS_PER_GRAPH : (j + 1) * ATOMS_PER_GRAPH]
                    if add_eng[t] == "scalar":
                        nc.scalar.add(out=blk, in_=blk, add=left[:, j : j + 1])
                    else:
                        nc.vector.tensor_scalar_add(
                            out=blk, in0=blk, scalar1=left[:, j : j + 1]
                        )
                if split_last and i == nt - 1:
                    hw_ = w_ // 2
                    nc.scalar.dma_start(
                        out=out_v[:, offs[t] : offs[t] + hw_], in_=x[:, 0:hw_]
                    )
                    nc.sync.dma_start(
                        out=out_v[:, offs[t] + hw_ : offs[t] + w_], in_=x[:, hw_:w_]
                    )
                else:
                    # stores for ACT-add tiles go on sync so they don't queue
                    # behind the ACT add chain; vector tiles use the ACT ring.
                    eng = nc.sync if add_eng[t] == "scalar" else nc.scalar
                    eng.dma_start(out=out_v[:, offs[t] : offs[t] + w_], in_=x[:])

    nc.finalize()
    return nc


_NC_CACHE = {}


def _get_nc():
    key = (TILE_W, TILE_ORDER, tuple(sorted(TILE_ADD_ENGINE.items())), RED_MODE,
           PSUM_W, SPLIT_LAST_STORE)
    if key not in _NC_CACHE:
        _NC_CACHE[key] = _build()
    return _NC_CACHE[key]


def _cpu_fallback(pc, total_charge, batch, n_atoms):
    num_segments = n_atoms.shape[0]
    seg = np.bincount(batch, weights=pc.astype(np.float64), minlength=num_segments)
    leftover = (total_charge - seg.astype(np.float32)) / n_atoms.astype(np.float32)
    return (pc + leftover[batch]).astype(np.float32)


_EYE = None


def kernel(**inputs) -> np.ndarray:
    global _EYE
    pc = np.ascontiguousarray(
        np.asarray(inputs["node_outputs"], dtype=np.float32).reshape(-1)
    )
    total_charge = np.ascontiguousarray(
        np.asarray(inputs["total_charge"], dtype=np.float32).reshape(-1)
    )
    batch = np.asarray(inputs["batch"]).reshape(-1)
    n_atoms = np.ascontiguousarray(np.asarray(inputs["n_atoms"], dtype=np.int32).reshape(-1))

    # The device kernel hardcodes the uniform 256-atoms-per-graph layout the
    # reference generator produces; anything else goes through numpy.
    if (
        pc.shape[0] != N_ATOMS
        or total_charge.shape[0] != N_GRAPHS
        or not np.all(n_atoms == ATOMS_PER_GRAPH)
        or not np.array_equal(
            batch.astype(np.int64),
            np.arange(N_ATOMS, dtype=np.int64) // ATOMS_PER_GRAPH,
        )
    ):
        return _cpu_fallback(pc, total_charge, batch, n_atoms)

    pc_b = pc.astype(ml_dtypes.bfloat16)
    tcs = (total_charge * (1.0 / ATOMS_PER_GRAPH)).astype(np.float32)
    if _EYE is None:
        _EYE = np.eye(P, dtype=ml_dtypes.bfloat16).reshape(-1)

    nc = _get_nc()
    in_maps = []
    for c in range(N_CORES):
        in_maps.append(
            {
                "pc": pc_b[c * A_PER_CORE : (c + 1) * A_PER_CORE],
                "tcs": tcs[c * G_PER_CORE : (c + 1) * G_PER_CORE],
                "eye": _EYE,
            }
        )
    res = run_bass_kernel_spmd(
        nc, in_maps, list(range(N_CORES)), trace=_TRACE, **_TRACE_KWARGS
    )
    out = np.concatenate([r["out"] for r in res.results]).astype(np.float32)
    if _TRACE:
        kernel.last_results = res
    return out


# revision 18
# speedup vs baseline: 1.0623x; 1.0588x over previous
"""Trainium2 Bass kernel for CorrectedPartialCharges.

out[i] = pc[i] + (total_charge[g] - seg_sum[g]) / n_atoms[g],  g = i // 256

Sharding: graphs are data-parallel across the 8 cores (4096 graphs /
1,048,576 atoms per core); segment sums and the gather-broadcast stay
device-local. On each core, partition p owns 32 contiguous graphs.

Wire format is bf16 (the 2e-2 rel-err budget allows it): node charges are
rounded to bf16 on the host, halving HBM traffic both ways; all device
accumulation is fp32. total_charge is pre-divided by 256 on the host so
the leftover is one fused scalar_tensor_tensor op.

Engine split (per [128, 2048] tile, k=8 graphs per partition):
  - Tensor: 4 accumulating identity matmuls fold each graph's 256 atoms
    into 64 PSUM columns (the bulk of the segment reduction, on an
    otherwise idle engine).
  - Vector: 1x reduce of the [128, k, 64] PSUM partials -> seg, fused
    leftover, and most per-graph tensor_scalar adds.
  - Scalar/ACT: per-graph bias adds for a subset of j-indices... no --
    whole-tile granularity (the tile dependency tracker serializes
    cross-engine writes to one tile), so ACT owns the adds of designated
    tiles and Vector the rest.
  - GpSimd: only the small identity + total-charge DMAs (SWDGE); its
    tensor ops have ~4.7us dispatch overhead and are useless here.
DMA: loads/stores alternate between the two HWDGE rings; the last store
is split across both rings to shorten the drain tail.
"""

import ml_dtypes
import numpy as np

import concourse.bacc as bacc
import concourse.bass as bass  # noqa: F401
import concourse.mybir as mybir
import concourse.tile as tile
from concourse.bass_utils import run_bass_kernel_spmd

N_CORES = 8
ATOMS_PER_GRAPH = 256
N_GRAPHS = 32768
N_ATOMS = N_GRAPHS * ATOMS_PER_GRAPH
P = 128

G_PER_CORE = N_GRAPHS // N_CORES          # 4096 graphs per core
A_PER_CORE = G_PER_CORE * ATOMS_PER_GRAPH  # 1,048,576 atoms per core

# Knobs read by test.py when experimenting.
# Tile widths by index (atoms per partition); tiles are contiguous spans of
# the free dim in index order. Sum must be 8192; each a multiple of 256 with
# width/256 <= 8 (PSUM accumulation bank limit).
TILE_W = (2048, 2048, 2048, 2048)
PSUM_W = 64               # columns per graph after the matmul pre-reduce
RED_MODE = "matmul"       # "matmul" | "halve"
TILE_ORDER = (2, 3, 0, 1)  # processing order; first listed loads first
# load ring per tile (alternating HWDGE rings in load order)
TILE_LOAD_Q = {2: "sync", 3: "scalar", 0: "sync", 1: "scalar"}
# add engine per tile ("vector" or "scalar"); ACT tiles should be early in
# TILE_ORDER so the ACT chain starts as soon as possible.
TILE_ADD_ENGINE = {2: "scalar", 3: "vector", 0: "vector", 1: "vector"}
SPLIT_LAST_STORE = True

_TRACE = False
_TRACE_KWARGS = {}


def _build(tile_w=None, order=None, add_eng=None, red_mode=None, psum_w=None,
           split_last=None):
    tile_w = TILE_W if tile_w is None else tile_w
    order = TILE_ORDER if order is None else order
    add_eng = TILE_ADD_ENGINE if add_eng is None else add_eng
    red_mode = RED_MODE if red_mode is None else red_mode
    psum_w = PSUM_W if psum_w is None else psum_w
    split_last = SPLIT_LAST_STORE if split_last is None else split_last

    nt = len(tile_w)
    ap_free = A_PER_CORE // P     # 8192 atoms per partition
    gp = G_PER_CORE // P          # 32 graphs per partition
    half = ATOMS_PER_GRAPH // 2
    n_pass = ATOMS_PER_GRAPH // psum_w
    offs = [0]
    for w_ in tile_w:
        offs.append(offs[-1] + w_)
    assert offs[-1] == ap_free
    for w_ in tile_w:
        assert w_ % ATOMS_PER_GRAPH == 0
        assert (w_ // ATOMS_PER_GRAPH) * psum_w * 4 <= 2048, \
            "psum accumulation group must fit one bank"
    assert tuple(sorted(order)) == tuple(range(nt))

    nc = bacc.Bacc(None, target_bir_lowering=False, enable_partition_id=False)

    pc = nc.dram_tensor("pc", [A_PER_CORE], mybir.dt.bfloat16, kind="ExternalInput")
    # total_charge / 256, fp32
    tcs = nc.dram_tensor("tcs", [G_PER_CORE], mybir.dt.float32, kind="ExternalInput")
    eye = nc.dram_tensor("eye", [P * P], mybir.dt.bfloat16, kind="ExternalInput")
    out = nc.dram_tensor("out", [A_PER_CORE], mybir.dt.bfloat16, kind="ExternalOutput")

    pc_v = pc[:].rearrange("(p n) -> p n", p=P)
    out_v = out[:].rearrange("(p n) -> p n", p=P)
    tcs_v = tcs[:].rearrange("(p k) -> p k", p=P)
    eye_v = eye[:].rearrange("(p n) -> p n", p=P)

    with tile.TileContext(nc) as tc:
        with (
            tc.tile_pool(name="io", bufs=nt) as io_pool,
            tc.tile_pool(name="half", bufs=2) as half_pool,
            tc.tile_pool(name="small", bufs=2 * nt) as small_pool,
            tc.tile_pool(name="consts", bufs=1) as const_pool,
            tc.tile_pool(name="psum", bufs=min(nt, 6), space="PSUM") as psum_pool,
        ):
            # Queue every input tile load up front on its assigned ring.
            xs = {}
            for i, t in enumerate(order):
                w_ = tile_w[t]
                x = io_pool.tile([P, w_], mybir.dt.bfloat16, tag="x")
                eng = getattr(nc, TILE_LOAD_Q[t])
                eng.dma_start(out=x[:], in_=pc_v[:, offs[t] : offs[t] + w_])
                xs[t] = x

            # Small constants ride the SWDGE (gpsimd) queue so they never
            # stall the HWDGE load rings (each extra DMA on a ring costs a
            # ~1.5us inter-DMA completion bubble).
            eye_tile = const_pool.tile([P, P], mybir.dt.bfloat16, tag="eye")
            nc.gpsimd.dma_start(out=eye_tile[:], in_=eye_v)
            tc_tile = const_pool.tile([P, gp], mybir.dt.float32, tag="tc")
            nc.gpsimd.dma_start(out=tc_tile[:], in_=tcs_v)

            goff = [o // ATOMS_PER_GRAPH for o in offs]  # graph offsets
            for i, t in enumerate(order):
                x = xs[t]
                w_ = tile_w[t]
                k = w_ // ATOMS_PER_GRAPH
                x3 = x[:].rearrange("p (k a) -> p k a", a=ATOMS_PER_GRAPH)

                seg = small_pool.tile([P, k], mybir.dt.float32, tag="seg")
                if red_mode == "matmul":
                    # Fold 256 atoms -> psum_w columns per graph with
                    # accumulating identity matmuls on the Tensor engine.
                    ps = psum_pool.tile([P, k, psum_w], mybir.dt.float32, tag="ps")
                    for s in range(n_pass):
                        nc.tensor.matmul(
                            ps[:],
                            eye_tile[:],
                            x3[:, :, s * psum_w : (s + 1) * psum_w],
                            start=(s == 0),
                            stop=(s == n_pass - 1),
                        )
                    nc.vector.reduce_sum(
                        out=seg[:], in_=ps[:], axis=mybir.AxisListType.X
                    )
                else:
                    u = half_pool.tile([P, k * half], mybir.dt.bfloat16, tag="u")
                    u3 = u[:].rearrange("p (k a) -> p k a", a=half)
                    nc.vector.tensor_add(
                        out=u3, in0=x3[:, :, 0:half],
                        in1=x3[:, :, half:ATOMS_PER_GRAPH],
                    )
                    nc.vector.reduce_sum(out=seg[:], in_=u3, axis=mybir.AxisListType.X)

                # left = (seg * -1/256) + tc/256   (fused)
                left = small_pool.tile([P, k], mybir.dt.float32, tag="left")
                nc.vector.scalar_tensor_tensor(
                    out=left[:],
                    in0=seg[:],
                    scalar=-1.0 / ATOMS_PER_GRAPH,
                    in1=tc_tile[:, goff[t] : goff[t] + k],
                    op0=mybir.AluOpType.mult,
                    op1=mybir.AluOpType.add,
                )

                for j in range(k):
                    blk = x[:, j * ATOMS_PER_GRAPH : (j + 1) * ATOMS_PER_GRAPH]
                    if add_eng[t] == "scalar":
                        nc.scalar.add(out=blk, in_=blk, add=left[:, j : j + 1])
                    else:
                        nc.vector.tensor_scalar_add(
                            out=blk, in0=blk, scalar1=left[:, j : j + 1]
                        )
                if split_last and i == nt - 1:
                    hw_ = w_ // 2
                    nc.scalar.dma_start(
                        out=out_v[:, offs[t] : offs[t] + hw_], in_=x[:, 0:hw_]
                    )
                    nc.sync.dma_start(
                        out=out_v[:, offs[t] + hw_ : offs[t] + w_], in_=x[:, hw_:w_]
                    )
                else:
                    # stores for ACT-add tiles go on sync so they don't queue
                    # behind the ACT add chain; vector tiles use the ACT ring.
                    eng = nc.sync if add_eng[t] == "scalar" else nc.scalar
                    eng.dma_start(out=out_v[:, offs[t] : offs[t] + w_], in_=x[:])

    nc.finalize()
    return nc


_NC_CACHE = {}


def _get_nc():
    key = (TILE_W, TILE_ORDER, tuple(sorted(TILE_ADD_ENGINE.items())), RED_MODE,
           PSUM_W, SPLIT_LAST_STORE)
    if key not in _NC_CACHE:
        _NC_CACHE[key] = _build()
    return _NC_CACHE[key]


def _cpu_fallback(pc, total_charge, batch, n_atoms):
    num_segments = n_atoms.shape[0]
    seg = np.bincount(batch, weights=pc.astype(np.float64), minlength=num_segments)
    leftover = (total_charge - seg.astype(np.float32)) / n_atoms.astype(np.float32)
    return (pc + leftover[batch]).astype(np.float32)


_EYE = None


def kernel(**inputs) -> np.ndarray:
    global _EYE
    pc = np.ascontiguousarray(
        np.asarray(inputs["node_outputs"], dtype=np.float32).reshape(-1)
    )
    total_charge = np.ascontiguousarray(
        np.asarray(inputs["total_charge"], dtype=np.float32).reshape(-1)
    )
    batch = np.asarray(inputs["batch"]).reshape(-1)
    n_atoms = np.ascontiguousarray(np.asarray(inputs["n_atoms"], dtype=np.int32).reshape(-1))

    # The device kernel hardcodes the uniform 256-atoms-per-graph layout the
    # reference generator produces; anything else goes through numpy.
    if (
        pc.shape[0] != N_ATOMS
        or total_charge.shape[0] != N_GRAPHS
        or not np.all(n_atoms == ATOMS_PER_GRAPH)
        or not np.array_equal(
            batch.astype(np.int64),
            np.arange(N_ATOMS, dtype=np.int64) // ATOMS_PER_GRAPH,
        )
    ):
        return _cpu_fallback(pc, total_charge, batch, n_atoms)

    pc_b = pc.astype(ml_dtypes.bfloat16)
    tcs = (total_charge * (1.0 / ATOMS_PER_GRAPH)).astype(np.float32)
    if _EYE is None:
        _EYE = np.eye(P, dtype=ml_dtypes.bfloat16).reshape(-1)

    nc = _get_nc()
    in_maps = []
    for c in range(N_CORES):
        in_maps.append(
            {
                "pc": pc_b[c * A_PER_CORE : (c + 1) * A_PER_CORE],
                "tcs": tcs[c * G_PER_CORE : (c + 1) * G_PER_CORE],
                "eye": _EYE,
            }
        )
    res = run_bass_kernel_spmd(
        nc, in_maps, list(range(N_CORES)), trace=_TRACE, **_TRACE_KWARGS
    )
    out = np.concatenate([r["out"] for r in res.results]).astype(np.float32)
    if _TRACE:
        kernel.last_results = res
    return out
